# revision 43
# baseline (speedup 1.0000x reference)
"""Trainium2 Bass kernel for the AbstractQCP residual operator F @ W.

Math (reference):
    v = y - s; mask = (v >= 0)
    dx = wx; dy = mask*wy; dt = wt        (W = [wx; wy; wt], (n+m+1, K))
    o1 = P@dx + A.T@dy + q dt             (n, K)
    o2 = b dt - A@dx                      (m, K)
    o3 = (x.T P x) dt - (q + 2 P x)@dx - b@dy
    F  = [o1; o2 + (1-mask)*wy; o3]       (since dx==wx, dt==wt the -dPi+W
                                           residual cancels on the n/t blocks)

Sharding across 8 NeuronCores (pure SPMD, no device collectives):
  core i owns output rows: o1[512i:512(i+1)], o2[1024i:1024(i+1)], and a
  partial of o3 (host sums the 8 (1,256) partials).
  GEMM1: lhsT_B = [P[:,cols_i]; A[:,cols_i]; q_i] (12289+pad, 512) -- P
  symmetric so P[:,cols] == P[rows,:].T.  rhs = [W | e] with e=[x;0;0]
  (257 cols) so column 256 of the GEMM1 result is P_i @ x for free.
  GEMM2: lhsT_C = [-A[rows_i,:].T; b_i] (4097+pad, 1024), rhs = n-block
  rows of W plus the wt row.
  All matmul operands bf16 (host-cast), accumulation fp32 in PSUM.

All streamed operands are staged in DRAM K-tile-transposed -- shape
(128, ktiles*free) with element (p, k*free+c) = orig(k*128+p, c) -- so a
single DMA moves several K-tiles with >=4KB contiguous per partition.
"""

import numpy as np
import ml_dtypes
from contextlib import ExitStack

BF = ml_dtypes.bfloat16

N, M, KP = 4096, 8192, 256
NC = 8
NS, MS = N // NC, M // NC          # 512, 1024
F = KP + 1                         # 257: probes + aug column
KT1, KT2 = 97, 33                  # contraction tiles (128 rows each)
R1, R2 = KT1 * 128, KT2 * 128      # 12416, 4224 (zero-padded)

G1 = 8     # wa / bt K-tiles per DMA group
G2 = 4     # ct K-tiles per DMA group

_NC_CACHE = None


def _kt(a, ktiles, free):
    """(ktiles*128, free) row-major -> (128, ktiles*free) K-tile-transposed."""
    return np.ascontiguousarray(
        a.reshape(ktiles, 128, free).transpose(1, 0, 2).reshape(128, ktiles * free))


def _build_nc():
    from concourse import bacc, tile, mybir
    from concourse.alu_op_type import AluOpType as op

    dtb = mybir.dt.bfloat16
    dtf = mybir.dt.float32

    nc = bacc.Bacc("TRN2", target_bir_lowering=False, debug=False)

    def din(name, shape, dt):
        return nc.dram_tensor(name, list(shape), dt, kind="ExternalInput").ap()

    bt = din("bt", (128, KT1 * NS), dtb)    # GEMM1 lhsT, K-tile-transposed
    ct = din("ct", (128, KT2 * MS), dtb)    # GEMM2 lhsT, K-tile-transposed
    wa = din("wa", (128, KT1 * F), dtb)     # [W | e] rhs, K-tile-transposed
    yt = din("yt", (128, 64), dtf)          # y in (partition, m-tile) layout
    st = din("st", (128, 64), dtf)
    yto = din("yto", (128, 8), dtf)         # own m-shard slices of yt/st
    sto = din("sto", (128, 8), dtf)
    nq = din("nq", (128, 4), dtf)           # -q_i
    xv = din("xv", (128, 4), dtb)           # x_i
    nb = din("nb", (128, 8), dtb)           # -b_i
    wown = din("wown", (128, 8 * F), dtb)   # own wy rows, K-tile-transposed
    xw = din("xw", (128, 4 * F), dtb)       # own wx rows, K-tile-transposed
    out1 = nc.dram_tensor("out1", [128, 4 * KP], dtb, kind="ExternalOutput").ap()
    out2 = nc.dram_tensor("out2", [128, 8 * KP], dtb, kind="ExternalOutput").ap()
    out3 = nc.dram_tensor("out3", [1, KP], dtf, kind="ExternalOutput").ap()

    with tile.TileContext(nc) as tc, ExitStack() as ctx:
        dpool = ctx.enter_context(tc.tile_pool(name="d", bufs=1))
        wpool = ctx.enter_context(tc.tile_pool(name="w", bufs=4))
        cpool = ctx.enter_context(tc.tile_pool(name="c", bufs=4))
        spool = ctx.enter_context(tc.tile_pool(name="s", bufs=1))
        opool = ctx.enter_context(tc.tile_pool(name="o", bufs=1))
        pspool = ctx.enter_context(tc.tile_pool(name="ps", bufs=8, space="PSUM"))

        # --- small vectors + masks: emitted as a deferred block (at the
        # end of loop step k==8) so their DMA triggers don't delay the
        # first weight chunks. None of them is consumed before ~k=24.
        sm = {}

        def emit_smalls():
            ytb = spool.tile((128, 64), dtf, tag="ytb")
            nc.scalar.dma_start(ytb, yt)
            stb = spool.tile((128, 64), dtf, tag="stb")
            nc.scalar.dma_start(stb, st)
            v = spool.tile((128, 64), dtf, tag="v")
            nc.vector.tensor_sub(v, ytb, stb)
            mask = spool.tile((128, 64), dtf, tag="mask")
            nc.vector.tensor_scalar(mask, v, 0.0, None, op.is_ge)
            sm["mask"] = mask

            ytob = spool.tile((128, 8), dtf, tag="ytob")
            nc.scalar.dma_start(ytob, yto)
            stob = spool.tile((128, 8), dtf, tag="stob")
            nc.scalar.dma_start(stob, sto)
            vo = spool.tile((128, 8), dtf, tag="vo")
            nc.vector.tensor_sub(vo, ytob, stob)
            masko = spool.tile((128, 8), dtf, tag="masko")
            nc.vector.tensor_scalar(masko, vo, 0.0, None, op.is_ge)
            umo = spool.tile((128, 8), dtf, tag="umo")
            nc.vector.tensor_scalar(umo, masko, -1.0, 1.0, op.mult, op.add)
            sm["umo"] = umo

            nqb = spool.tile((128, 4), dtf, tag="nqb")
            nc.scalar.dma_start(nqb, nq)
            sm["nqb"] = nqb
            xvb = spool.tile((128, 4), dtb, tag="xvb")
            nc.scalar.dma_start(xvb, xv)
            sm["xvb"] = xvb
            nbb = spool.tile((128, 8), dtb, tag="nbb")
            nc.scalar.dma_start(nbb, nb)
            sm["nbb"] = nbb

            wosb = spool.tile((128, 8 * F), dtb, tag="wosb")
            nc.scalar.dma_start(wosb, wown)
            sm["wosb"] = wosb
            wm = []
            for t_i in range(8):
                mt = spool.tile((128, F), dtb, tag=f"wm{t_i}")
                nc.vector.tensor_scalar_mul(mt, wosb[:, t_i * F:(t_i + 1) * F],
                                            masko[:, t_i:t_i + 1])
                wm.append(mt)
            sm["wm"] = wm
            xwsb = spool.tile((128, 4 * F), dtb, tag="xwsb")
            nc.scalar.dma_start(xwsb, xw)
            sm["xwsb"] = xwsb

        # --- resident rhs tiles; group DMAs are emitted just-in-time
        # inside the unified loop. First groups are small so the PE's
        # first inputs land quickly after the preamble.
        WB = [0, 1, 2, 4, 8, 16, 24, 32, 40, 48, 56, 64, 72, 80, 88, 96, 97]
        BB = [0, 1, 2, 4, 8, 16, 24, 32, 40, 48, 56, 64, 72, 80, 88, 96, 97]  # bt chunk bounds
        CB = [0, 1, 4, 8, 12, 16, 20, 24, 28, 32, 33]
        k2g = {}
        for g in range(len(WB) - 1):
            for k in range(WB[g], WB[g + 1]):
                k2g[k] = g
        k2b = {}
        for g in range(len(BB) - 1):
            for k in range(BB[g], BB[g + 1]):
                k2b[k] = g
        j2c = {}
        for c in range(len(CB) - 1):
            for j in range(CB[c], CB[c + 1]):
                j2c[j] = c

        wag = [None] * (len(WB) - 1)
        dm = [None] * 64

        def load_wag(g):
            k0, k1 = WB[g], WB[g + 1]
            t = dpool.tile((128, (k1 - k0) * F), dtb, tag=f"wag{g}", name=f"wag{g}")
            nc.sync.dma_start(t, wa[:, k0 * F:k1 * F])
            wag[g] = t
            for j in range(k0, k1):
                jm = j - 32
                if 0 <= jm < 64:
                    mt = dpool.tile((128, F), dtb, tag=f"dm{jm}", name=f"dm{jm}")
                    nc.vector.tensor_scalar_mul(
                        mt, t[:, (j - k0) * F:(j - k0 + 1) * F], sm["mask"][:, jm:jm + 1])
                    dm[jm] = mt

        def dslice(k):
            g = k2g[k]
            return wag[g][:, (k - WB[g]) * F:(k - WB[g] + 1) * F]

        def rhs1(k):
            return dslice(k) if (k < 32 or k == 96) else dm[k - 32]

        # --- unified interleaved loop ---------------------------------
        # GEMM1 k-tile per step; GEMM2 tick j at step 5+round(2.3j) --
        # sparse early (while wa streams) and stopping at step 79 so the
        # f2 evictions + output DMA overlap the last GEMM1 steps.
        # psum: gemm1 4 banks (128,257); gemm2 4 banks (128,512) holding
        # two 256-wide accumulators each (bank-shared start/stop flags).
        ps1 = [pspool.tile((128, F), dtf, tag="ps", name=f"ps1_{m}") for m in range(4)]
        ps2 = [pspool.tile((128, 512), dtf, tag="ps", name=f"ps2_{u}") for u in range(4)]

        tick_at = {(0 if j == 0 else 13 + round(1.9 * j)): j for j in range(KT2)}

        load_wag(0)
        btt = None
        btt_k0 = 0
        ctt = None
        ctt_j0 = 0
        px = spool.tile((128, 4), dtb, tag="px")
        cf = spool.tile((128, 4), dtb, tag="cf")
        pso3 = None
        for k in range(KT1):
            if k == 0 or k2g[k] != k2g[k - 1]:
                g = k2g[k]
                if g + 1 < len(WB) - 2:
                    load_wag(g + 1)
            if k == 0 or k2b[k] != k2b[k - 1]:
                g = k2b[k]
                k0, k1 = BB[g], BB[g + 1]
                btt = wpool.tile((128, G1 * NS), dtb, tag="bt",
                                 name=f"btt{g}", padded_shape=(128, G1 * NS))
                btt_k0 = k0
                nc.sync.dma_start(btt[:, :(k1 - k0) * NS], bt[:, k0 * NS:k1 * NS])
            if k == 40:
                load_wag(len(WB) - 2)   # d[96] (wt row): needed at tick 32
            j = tick_at.get(k)
            if j is not None:
                if j == 0 or j2c[j] != j2c[j - 1]:
                    c = j2c[j]
                    j0, j1 = CB[c], CB[c + 1]
                    ctt = cpool.tile((128, G2 * MS), dtb, tag="ct",
                                     name=f"ctt{c}", padded_shape=(128, G2 * MS))
                    ctt_j0 = j0
                    nc.gpsimd.dma_start(ctt[:, :(j1 - j0) * MS], ct[:, j0 * MS:j1 * MS])
                rhs2 = dslice(j)[:, 0:KP] if j < 32 else dslice(96)[:, 0:KP]
                jo = j - ctt_j0
                for t_i in range(8):
                    # Bank sharing: slice t%2==0 owns start (clears whole
                    # bank); slice t%2==1's first write lands on cleared
                    # has_written bits so flags=0 overwrites correctly.
                    # Only the bank's last write carries stop.
                    nc.tensor.matmul(
                        ps2[t_i // 2][:, (t_i % 2) * KP:(t_i % 2 + 1) * KP],
                        ctt[:, jo * MS + t_i * 128:jo * MS + (t_i + 1) * 128],
                        rhs2, start=(j == 0 and t_i % 2 == 0),
                        stop=(j == KT2 - 1 and t_i % 2 == 1))
                if j == KT2 - 1:
                    # gemm2 done: evict f2 while gemm1 finishes; stage all
                    # 8 tiles contiguously so one DMA moves 8KB/partition
                    ob2 = opool.tile((128, 8 * KP), dtb, tag="ob2")
                    for t_i in range(8):
                        # f2 = (wy * (1-mask)) + o2pre
                        nc.vector.scalar_tensor_tensor(
                            ob2[:, t_i * KP:(t_i + 1) * KP],
                            sm["wosb"][:, t_i * F:t_i * F + KP], sm["umo"][:, t_i:t_i + 1],
                            ps2[t_i // 2][:, (t_i % 2) * KP:(t_i % 2 + 1) * KP],
                            op.mult, op.add)
                    nc.gpsimd.dma_start(out2, ob2)
            if k == 88:
                # o3's -b@dy half: no gemm1 dependency, runs while psum
                # slots freed by the f2 evictions are available
                pso3 = pspool.tile((1, F), dtf, tag="ps", name="pso3")
                for t_i in range(8):
                    nc.tensor.matmul(pso3, sm["nbb"][:, t_i:t_i + 1], sm["wm"][t_i],
                                     start=(t_i == 0), stop=False)
            if k == 81:
                # phase2 accumulators take the banks gemm2 just freed
                ps1b = [pspool.tile((128, KP), dtf, tag="ps", name=f"ps1b_{m}")
                        for m in range(4)]
            jb = k - btt_k0
            for m in range(4):
                if k <= 80:
                    nc.tensor.matmul(
                        ps1[m],
                        btt[:, jb * NS + m * 128:jb * NS + (m + 1) * 128],
                        rhs1(k), start=(k == 0), stop=(k == 80))
                else:
                    nc.tensor.matmul(
                        ps1b[m],
                        btt[:, jb * NS + m * 128:jb * NS + (m + 1) * 128],
                        rhs1(k)[:, 0:KP], start=(k == 81), stop=(k == KT1 - 1))
            if k == 16:
                emit_smalls()
            if k == 80:
                # phase1 eviction mid-loop: Px column is complete (aug col
                # is zero past the n block), so the whole o3 chain can run
                # inside the loop
                pr = []
                for m in range(4):
                    nc.vector.tensor_copy(px[:, m:m + 1], ps1[m][:, KP:KP + 1])
                    # cf = -(q + 2 Px) = (Px * -2) + (-q)
                    nc.vector.scalar_tensor_tensor(
                        cf[:, m:m + 1], ps1[m][:, KP:KP + 1], -2.0,
                        sm["nqb"][:, m:m + 1], op.mult, op.add)
                for m in range(4):
                    t = spool.tile((128, KP), dtf, tag=f"pr{m}")
                    nc.vector.tensor_copy(t, ps1[m][:, 0:KP])
                    pr.append(t)
            if k == 86:
                psxx = pspool.tile((1, 1), dtf, tag="ps")
                for j3 in range(4):
                    nc.tensor.matmul(psxx, px[:, j3:j3 + 1], sm["xvb"][:, j3:j3 + 1],
                                     start=(j3 == 0), stop=(j3 == 3))
            if k == 90:
                for j3 in range(4):
                    nc.tensor.matmul(pso3, cf[:, j3:j3 + 1],
                                     sm["xwsb"][:, j3 * F:(j3 + 1) * F],
                                     start=False, stop=(j3 == 3))
            if k == 92:
                o3f = opool.tile((1, KP), dtf, tag="o3f")
                # o3 = wt * xPx + (cf@dx + (-b)@dy)
                nc.vector.scalar_tensor_tensor(o3f, dslice(96)[0:1, 0:KP],
                                               psxx[0:1, 0:1], pso3[0:1, 0:KP],
                                               op.mult, op.add)
                nc.scalar.dma_start(out3, o3f)

        # --- final combine: o1 = phase1 partial + phase2 psum ---------
        ob1 = opool.tile((128, 4 * KP), dtb, tag="ob1")
        for m in range(4):
            nc.vector.tensor_tensor(ob1[:, m * KP:(m + 1) * KP], pr[m],
                                    ps1b[m][:, 0:KP], op.add)
        nc.gpsimd.dma_start(out1, ob1)

    nc.compile()
    return nc


def _get_nc():
    global _NC_CACHE
    if _NC_CACHE is None:
        _NC_CACHE = _build_nc()
    return _NC_CACHE


def _prep_in_maps(P, A, q, b, x, y, s, W):
    P = np.asarray(P, np.float32)
    A = np.asarray(A, np.float32)
    q = np.asarray(q, np.float32)
    b = np.asarray(b, np.float32)
    x = np.asarray(x, np.float32)
    y = np.asarray(y, np.float32)
    s = np.asarray(s, np.float32)
    W = np.asarray(W, np.float32)

    Pb, Ab = P.astype(BF), A.astype(BF)
    qb, bb, xb, Wb = q.astype(BF), b.astype(BF), x.astype(BF), W.astype(BF)

    wa0 = np.zeros((R1, F), BF)
    wa0[:N + M + 1, :KP] = Wb
    wa0[:N, KP] = xb
    wa = _kt(wa0, KT1, F)
    yt = np.ascontiguousarray(y.reshape(64, 128).T)
    st_ = np.ascontiguousarray(s.reshape(64, 128).T)

    in_maps = []
    for i in range(NC):
        ncol = slice(i * NS, (i + 1) * NS)
        mrow = slice(i * MS, (i + 1) * MS)
        bt0 = np.zeros((R1, NS), BF)
        bt0[:N] = Pb[:, ncol]
        bt0[N:N + M] = Ab[:, ncol]
        bt0[N + M] = qb[ncol]
        ct0 = np.zeros((R2, MS), BF)
        ct0[:N] = (-A[mrow].T).astype(BF)
        ct0[N] = bb[mrow]
        in_maps.append(dict(
            bt=_kt(bt0, KT1, NS), ct=_kt(ct0, KT2, MS), wa=wa,
            yt=yt, st=st_,
            yto=np.ascontiguousarray(yt[:, 8 * i:8 * i + 8]),
            sto=np.ascontiguousarray(st_[:, 8 * i:8 * i + 8]),
            nq=np.ascontiguousarray((-q[ncol]).reshape(4, 128).T),
            xv=np.ascontiguousarray(x[ncol].reshape(4, 128).T.astype(BF)),
            nb=np.ascontiguousarray((-b[mrow]).reshape(8, 128).T.astype(BF)),
            wown=_kt(wa0[N + i * MS:N + (i + 1) * MS], 8, F),
            xw=_kt(wa0[i * NS:(i + 1) * NS], 4, F),
        ))
    return in_maps


def _assemble(results):
    Fo = np.empty((N + M + 1, KP), np.float32)
    o3 = np.zeros((KP,), np.float32)
    for i in range(NC):
        o1 = np.asarray(results[i]["out1"], np.float32)     # (128, 4*KP)
        o2 = np.asarray(results[i]["out2"], np.float32)     # (128, 8*KP)
        Fo[i * NS:(i + 1) * NS] = (
            o1.reshape(128, 4, KP).transpose(1, 0, 2).reshape(NS, KP))
        Fo[N + i * MS:N + (i + 1) * MS] = (
            o2.reshape(128, 8, KP).transpose(1, 0, 2).reshape(MS, KP))
        o3 += np.asarray(results[i]["out3"], np.float32)[0]
    Fo[N + M] = o3
    return Fo


def _run_sharded(inputs, trace=False, trace_kwargs=None):
    from concourse import bass_utils
    nc = _get_nc()
    in_maps = _prep_in_maps(**inputs)
    res = bass_utils.run_bass_kernel_spmd(
        nc, in_maps, core_ids=list(range(NC)), trace=trace,
        **(trace_kwargs or {}))
    return _assemble(res.results), res


def kernel(**inputs) -> np.ndarray:
    out, _ = _run_sharded(inputs, trace=False)
    return out


# revision 44
# speedup vs baseline: 1.0736x; 1.0736x over previous
"""Trainium2 Bass kernel for the AbstractQCP residual operator F @ W.

Math (reference):
    v = y - s; mask = (v >= 0)
    dx = wx; dy = mask*wy; dt = wt        (W = [wx; wy; wt], (n+m+1, K))
    o1 = P@dx + A.T@dy + q dt             (n, K)
    o2 = b dt - A@dx                      (m, K)
    o3 = (x.T P x) dt - (q + 2 P x)@dx - b@dy
    F  = [o1; o2 + (1-mask)*wy; o3]       (since dx==wx, dt==wt the -dPi+W
                                           residual cancels on the n/t blocks)

Sharding across 8 NeuronCores (pure SPMD, no device collectives):
  core i owns output rows: o1[512i:512(i+1)], o2[1024i:1024(i+1)], and a
  partial of o3 (host sums the 8 (1,256) partials).
  GEMM1: lhsT_B = [P[:,cols_i]; A[:,cols_i]; q_i] (12289+pad, 512) -- P
  symmetric so P[:,cols] == P[rows,:].T.  rhs = [W | e] with e=[x;0;0]
  (257 cols) so column 256 of the GEMM1 result is P_i @ x for free.
  GEMM2: lhsT_C = [-A[rows_i,:].T; b_i] (4097+pad, 1024), rhs = n-block
  rows of W plus the wt row.
  All matmul operands bf16 (host-cast), accumulation fp32 in PSUM.

All streamed operands are staged in DRAM K-tile-transposed -- shape
(128, ktiles*free) with element (p, k*free+c) = orig(k*128+p, c) -- so a
single DMA moves several K-tiles with >=4KB contiguous per partition.
"""

import numpy as np
import ml_dtypes
from contextlib import ExitStack

BF = ml_dtypes.bfloat16

N, M, KP = 4096, 8192, 256
NC = 8
NS, MS = N // NC, M // NC          # 512, 1024
F = KP + 1                         # 257: probes + aug column
KT1, KT2 = 97, 33                  # contraction tiles (128 rows each)
R1, R2 = KT1 * 128, KT2 * 128      # 12416, 4224 (zero-padded)

G1 = 8     # wa / bt K-tiles per DMA group
G2 = 4     # ct K-tiles per DMA group

_NC_CACHE = None


def _kt(a, ktiles, free):
    """(ktiles*128, free) row-major -> (128, ktiles*free) K-tile-transposed."""
    return np.ascontiguousarray(
        a.reshape(ktiles, 128, free).transpose(1, 0, 2).reshape(128, ktiles * free))


def _build_nc():
    from concourse import bacc, tile, mybir
    from concourse.alu_op_type import AluOpType as op

    dtb = mybir.dt.bfloat16
    dtf = mybir.dt.float32

    nc = bacc.Bacc("TRN2", target_bir_lowering=False, debug=False)

    def din(name, shape, dt):
        return nc.dram_tensor(name, list(shape), dt, kind="ExternalInput").ap()

    bt = din("bt", (128, KT1 * NS), dtb)    # GEMM1 lhsT, K-tile-transposed
    ct = din("ct", (128, KT2 * MS), dtb)    # GEMM2 lhsT, K-tile-transposed
    wa = din("wa", (128, KT1 * F), dtb)     # [W | e] rhs, K-tile-transposed
    yt = din("yt", (128, 64), dtf)          # y in (partition, m-tile) layout
    st = din("st", (128, 64), dtf)
    yto = din("yto", (128, 8), dtf)         # own m-shard slices of yt/st
    sto = din("sto", (128, 8), dtf)
    nq = din("nq", (128, 4), dtf)           # -q_i
    xv = din("xv", (128, 4), dtb)           # x_i
    nb = din("nb", (128, 8), dtb)           # -b_i
    wown = din("wown", (128, 8 * F), dtb)   # own wy rows, K-tile-transposed
    xw = din("xw", (128, 4 * F), dtb)       # own wx rows, K-tile-transposed
    out1 = nc.dram_tensor("out1", [128, 4 * KP], dtb, kind="ExternalOutput").ap()
    out2 = nc.dram_tensor("out2", [128, 8 * KP], dtb, kind="ExternalOutput").ap()
    out3 = nc.dram_tensor("out3", [1, KP], dtf, kind="ExternalOutput").ap()

    with tile.TileContext(nc) as tc, ExitStack() as ctx:
        dpool = ctx.enter_context(tc.tile_pool(name="d", bufs=1))
        wpool = ctx.enter_context(tc.tile_pool(name="w", bufs=4))
        cpool = ctx.enter_context(tc.tile_pool(name="c", bufs=4))
        spool = ctx.enter_context(tc.tile_pool(name="s", bufs=1))
        opool = ctx.enter_context(tc.tile_pool(name="o", bufs=1))
        pspool = ctx.enter_context(tc.tile_pool(name="ps", bufs=8, space="PSUM"))

        # --- small vectors + masks: emitted as a deferred block (at the
        # end of loop step k==8) so their DMA triggers don't delay the
        # first weight chunks. None of them is consumed before ~k=24.
        sm = {}

        def emit_smalls():
            ytb = spool.tile((128, 64), dtf, tag="ytb")
            nc.scalar.dma_start(ytb, yt)
            stb = spool.tile((128, 64), dtf, tag="stb")
            nc.scalar.dma_start(stb, st)
            v = spool.tile((128, 64), dtf, tag="v")
            nc.vector.tensor_sub(v, ytb, stb)
            mask = spool.tile((128, 64), dtf, tag="mask")
            nc.vector.tensor_scalar(mask, v, 0.0, None, op.is_ge)
            sm["mask"] = mask

            ytob = spool.tile((128, 8), dtf, tag="ytob")
            nc.scalar.dma_start(ytob, yto)
            stob = spool.tile((128, 8), dtf, tag="stob")
            nc.scalar.dma_start(stob, sto)
            vo = spool.tile((128, 8), dtf, tag="vo")
            nc.vector.tensor_sub(vo, ytob, stob)
            masko = spool.tile((128, 8), dtf, tag="masko")
            nc.vector.tensor_scalar(masko, vo, 0.0, None, op.is_ge)
            umo = spool.tile((128, 8), dtf, tag="umo")
            nc.vector.tensor_scalar(umo, masko, -1.0, 1.0, op.mult, op.add)
            sm["umo"] = umo

            nqb = spool.tile((128, 4), dtf, tag="nqb")
            nc.scalar.dma_start(nqb, nq)
            sm["nqb"] = nqb
            xvb = spool.tile((128, 4), dtb, tag="xvb")
            nc.scalar.dma_start(xvb, xv)
            sm["xvb"] = xvb
            nbb = spool.tile((128, 8), dtb, tag="nbb")
            nc.scalar.dma_start(nbb, nb)
            sm["nbb"] = nbb

            wosb = spool.tile((128, 8 * F), dtb, tag="wosb")
            nc.scalar.dma_start(wosb, wown)
            sm["wosb"] = wosb
            wm = []
            for t_i in range(8):
                mt = spool.tile((128, F), dtb, tag=f"wm{t_i}")
                nc.vector.tensor_scalar_mul(mt, wosb[:, t_i * F:(t_i + 1) * F],
                                            masko[:, t_i:t_i + 1])
                wm.append(mt)
            sm["wm"] = wm
            xwsb = spool.tile((128, 4 * F), dtb, tag="xwsb")
            nc.scalar.dma_start(xwsb, xw)
            sm["xwsb"] = xwsb

        # --- resident rhs tiles; group DMAs are emitted just-in-time
        # inside the unified loop. First groups are small so the PE's
        # first inputs land quickly after the preamble.
        WB = [0, 1, 2, 4, 8, 16, 24, 32, 40, 48, 56, 64, 72, 80, 88, 96, 97]
        BB = [0, 1, 2, 4, 8, 16, 24, 32, 40, 48, 56, 64, 72, 80, 88, 96, 97]  # bt chunk bounds
        CB = [0, 1, 4, 8, 12, 16, 20, 24, 28, 32, 33]
        k2g = {}
        for g in range(len(WB) - 1):
            for k in range(WB[g], WB[g + 1]):
                k2g[k] = g
        k2b = {}
        for g in range(len(BB) - 1):
            for k in range(BB[g], BB[g + 1]):
                k2b[k] = g
        j2c = {}
        for c in range(len(CB) - 1):
            for j in range(CB[c], CB[c + 1]):
                j2c[j] = c

        wag = [None] * (len(WB) - 1)
        dm = [None] * 64

        def load_wag(g):
            k0, k1 = WB[g], WB[g + 1]
            t = dpool.tile((128, (k1 - k0) * F), dtb, tag=f"wag{g}", name=f"wag{g}")
            nc.sync.dma_start(t, wa[:, k0 * F:k1 * F])
            wag[g] = t
            for j in range(k0, k1):
                jm = j - 32
                if 0 <= jm < 64:
                    mt = dpool.tile((128, F), dtb, tag=f"dm{jm}", name=f"dm{jm}")
                    nc.vector.tensor_scalar_mul(
                        mt, t[:, (j - k0) * F:(j - k0 + 1) * F], sm["mask"][:, jm:jm + 1])
                    dm[jm] = mt

        def dslice(k):
            g = k2g[k]
            return wag[g][:, (k - WB[g]) * F:(k - WB[g] + 1) * F]

        def rhs1(k):
            return dslice(k) if (k < 32 or k == 96) else dm[k - 32]

        # --- unified interleaved loop ---------------------------------
        # GEMM1 k-tile per step; GEMM2 tick j at step 5+round(2.3j) --
        # sparse early (while wa streams) and stopping at step 79 so the
        # f2 evictions + output DMA overlap the last GEMM1 steps.
        # psum: gemm1 4 banks (128,257); gemm2 4 banks (128,512) holding
        # two 256-wide accumulators each (bank-shared start/stop flags).
        ps1 = [pspool.tile((128, F), dtf, tag="ps", name=f"ps1_{m}") for m in range(4)]
        ps2 = [pspool.tile((128, 512), dtf, tag="ps", name=f"ps2_{u}") for u in range(4)]

        tick_at = {(0 if j == 0 else 12 + round(1.94 * j)): j for j in range(KT2)}

        load_wag(0)
        btt = None
        btt_k0 = 0
        ctt = None
        ctt_j0 = 0
        px = spool.tile((128, 4), dtb, tag="px")
        cf = spool.tile((128, 4), dtb, tag="cf")
        pso3 = None
        for k in range(KT1):
            if k == 0 or k2g[k] != k2g[k - 1]:
                g = k2g[k]
                if g + 1 < len(WB) - 2:
                    load_wag(g + 1)
            if k == 0 or k2b[k] != k2b[k - 1]:
                g = k2b[k]
                k0, k1 = BB[g], BB[g + 1]
                btt = wpool.tile((128, G1 * NS), dtb, tag="bt",
                                 name=f"btt{g}", padded_shape=(128, G1 * NS))
                btt_k0 = k0
                nc.sync.dma_start(btt[:, :(k1 - k0) * NS], bt[:, k0 * NS:k1 * NS])
            if k == 40:
                load_wag(len(WB) - 2)   # d[96] (wt row): needed at tick 32
            j = tick_at.get(k)
            if j is not None:
                if j == 0 or j2c[j] != j2c[j - 1]:
                    c = j2c[j]
                    j0, j1 = CB[c], CB[c + 1]
                    ctt = cpool.tile((128, G2 * MS), dtb, tag="ct",
                                     name=f"ctt{c}", padded_shape=(128, G2 * MS))
                    ctt_j0 = j0
                    nc.gpsimd.dma_start(ctt[:, :(j1 - j0) * MS], ct[:, j0 * MS:j1 * MS])
                rhs2 = dslice(j)[:, 0:KP] if j < 32 else dslice(96)[:, 0:KP]
                jo = j - ctt_j0
                for t_i in range(8):
                    # Bank sharing: slice t%2==0 owns start (clears whole
                    # bank); slice t%2==1's first write lands on cleared
                    # has_written bits so flags=0 overwrites correctly.
                    # Only the bank's last write carries stop.
                    nc.tensor.matmul(
                        ps2[t_i // 2][:, (t_i % 2) * KP:(t_i % 2 + 1) * KP],
                        ctt[:, jo * MS + t_i * 128:jo * MS + (t_i + 1) * 128],
                        rhs2, start=(j == 0 and t_i % 2 == 0),
                        stop=(j == KT2 - 1 and t_i % 2 == 1))
                if j == KT2 - 1:
                    # gemm2 done: evict f2 while gemm1 finishes; stage all
                    # 8 tiles contiguously so one DMA moves 8KB/partition
                    ob2 = opool.tile((128, 8 * KP), dtb, tag="ob2")
                    for t_i in range(8):
                        # f2 = (wy * (1-mask)) + o2pre
                        nc.vector.scalar_tensor_tensor(
                            ob2[:, t_i * KP:(t_i + 1) * KP],
                            sm["wosb"][:, t_i * F:t_i * F + KP], sm["umo"][:, t_i:t_i + 1],
                            ps2[t_i // 2][:, (t_i % 2) * KP:(t_i % 2 + 1) * KP],
                            op.mult, op.add)
                    nc.scalar.dma_start(out2, ob2)
            if k == 88:
                # o3's -b@dy half: no gemm1 dependency, runs while psum
                # slots freed by the f2 evictions are available
                pso3 = pspool.tile((1, F), dtf, tag="ps", name="pso3")
                for t_i in range(8):
                    nc.tensor.matmul(pso3, sm["nbb"][:, t_i:t_i + 1], sm["wm"][t_i],
                                     start=(t_i == 0), stop=False)
            if k == 81:
                # phase2 accumulators take the banks gemm2 just freed
                ps1b = [pspool.tile((128, KP), dtf, tag="ps", name=f"ps1b_{m}")
                        for m in range(4)]
            jb = k - btt_k0
            for m in range(4):
                if k <= 80:
                    nc.tensor.matmul(
                        ps1[m],
                        btt[:, jb * NS + m * 128:jb * NS + (m + 1) * 128],
                        rhs1(k), start=(k == 0), stop=(k == 80))
                else:
                    nc.tensor.matmul(
                        ps1b[m],
                        btt[:, jb * NS + m * 128:jb * NS + (m + 1) * 128],
                        rhs1(k)[:, 0:KP], start=(k == 81), stop=(k == KT1 - 1))
            if k == 16:
                emit_smalls()
            if k == 80:
                # phase1 eviction mid-loop: Px column is complete (aug col
                # is zero past the n block), so the whole o3 chain can run
                # inside the loop
                pr = []
                for m in range(4):
                    nc.vector.tensor_copy(px[:, m:m + 1], ps1[m][:, KP:KP + 1])
                    # cf = -(q + 2 Px) = (Px * -2) + (-q)
                    nc.vector.scalar_tensor_tensor(
                        cf[:, m:m + 1], ps1[m][:, KP:KP + 1], -2.0,
                        sm["nqb"][:, m:m + 1], op.mult, op.add)
                for m in range(4):
                    t = spool.tile((128, KP), dtf, tag=f"pr{m}")
                    nc.vector.tensor_copy(t, ps1[m][:, 0:KP])
                    pr.append(t)
            if k == 86:
                psxx = pspool.tile((1, 1), dtf, tag="ps")
                for j3 in range(4):
                    nc.tensor.matmul(psxx, px[:, j3:j3 + 1], sm["xvb"][:, j3:j3 + 1],
                                     start=(j3 == 0), stop=(j3 == 3))
            if k == 90:
                for j3 in range(4):
                    nc.tensor.matmul(pso3, cf[:, j3:j3 + 1],
                                     sm["xwsb"][:, j3 * F:(j3 + 1) * F],
                                     start=False, stop=(j3 == 3))
            if k == 92:
                o3f = opool.tile((1, KP), dtf, tag="o3f")
                # o3 = wt * xPx + (cf@dx + (-b)@dy)
                nc.vector.scalar_tensor_tensor(o3f, dslice(96)[0:1, 0:KP],
                                               psxx[0:1, 0:1], pso3[0:1, 0:KP],
                                               op.mult, op.add)
                nc.scalar.dma_start(out3, o3f)

        # --- final combine: o1 = phase1 partial + phase2 psum ---------
        ob1 = opool.tile((128, 4 * KP), dtb, tag="ob1")
        for m in range(4):
            nc.vector.tensor_tensor(ob1[:, m * KP:(m + 1) * KP], pr[m],
                                    ps1b[m][:, 0:KP], op.add)
        nc.scalar.dma_start(out1, ob1)

    nc.compile()
    return nc


def _get_nc():
    global _NC_CACHE
    if _NC_CACHE is None:
        _NC_CACHE = _build_nc()
    return _NC_CACHE


def _prep_in_maps(P, A, q, b, x, y, s, W):
    P = np.asarray(P, np.float32)
    A = np.asarray(A, np.float32)
    q = np.asarray(q, np.float32)
    b = np.asarray(b, np.float32)
    x = np.asarray(x, np.float32)
    y = np.asarray(y, np.float32)
    s = np.asarray(s, np.float32)
    W = np.asarray(W, np.float32)

    Pb, Ab = P.astype(BF), A.astype(BF)
    qb, bb, xb, Wb = q.astype(BF), b.astype(BF), x.astype(BF), W.astype(BF)

    wa0 = np.zeros((R1, F), BF)
    wa0[:N + M + 1, :KP] = Wb
    wa0[:N, KP] = xb
    wa = _kt(wa0, KT1, F)
    yt = np.ascontiguousarray(y.reshape(64, 128).T)
    st_ = np.ascontiguousarray(s.reshape(64, 128).T)

    in_maps = []
    for i in range(NC):
        ncol = slice(i * NS, (i + 1) * NS)
        mrow = slice(i * MS, (i + 1) * MS)
        bt0 = np.zeros((R1, NS), BF)
        bt0[:N] = Pb[:, ncol]
        bt0[N:N + M] = Ab[:, ncol]
        bt0[N + M] = qb[ncol]
        ct0 = np.zeros((R2, MS), BF)
        ct0[:N] = (-A[mrow].T).astype(BF)
        ct0[N] = bb[mrow]
        in_maps.append(dict(
            bt=_kt(bt0, KT1, NS), ct=_kt(ct0, KT2, MS), wa=wa,
            yt=yt, st=st_,
            yto=np.ascontiguousarray(yt[:, 8 * i:8 * i + 8]),
            sto=np.ascontiguousarray(st_[:, 8 * i:8 * i + 8]),
            nq=np.ascontiguousarray((-q[ncol]).reshape(4, 128).T),
            xv=np.ascontiguousarray(x[ncol].reshape(4, 128).T.astype(BF)),
            nb=np.ascontiguousarray((-b[mrow]).reshape(8, 128).T.astype(BF)),
            wown=_kt(wa0[N + i * MS:N + (i + 1) * MS], 8, F),
            xw=_kt(wa0[i * NS:(i + 1) * NS], 4, F),
        ))
    return in_maps


def _assemble(results):
    Fo = np.empty((N + M + 1, KP), np.float32)
    o3 = np.zeros((KP,), np.float32)
    for i in range(NC):
        o1 = np.asarray(results[i]["out1"], np.float32)     # (128, 4*KP)
        o2 = np.asarray(results[i]["out2"], np.float32)     # (128, 8*KP)
        Fo[i * NS:(i + 1) * NS] = (
            o1.reshape(128, 4, KP).transpose(1, 0, 2).reshape(NS, KP))
        Fo[N + i * MS:N + (i + 1) * MS] = (
            o2.reshape(128, 8, KP).transpose(1, 0, 2).reshape(MS, KP))
        o3 += np.asarray(results[i]["out3"], np.float32)[0]
    Fo[N + M] = o3
    return Fo


def _run_sharded(inputs, trace=False, trace_kwargs=None):
    from concourse import bass_utils
    nc = _get_nc()
    in_maps = _prep_in_maps(**inputs)
    res = bass_utils.run_bass_kernel_spmd(
        nc, in_maps, core_ids=list(range(NC)), trace=trace,
        **(trace_kwargs or {}))
    return _assemble(res.results), res


def kernel(**inputs) -> np.ndarray:
    out, _ = _run_sharded(inputs, trace=False)
    return out
